# revision 30
# baseline (speedup 1.0000x reference)
"""Trainium2 Bass kernel for the SSD-style detection loss (nn_DetectionLoss).

v3 design — exploits the separable SSD anchor grid:
  inter[h, (a,w)] for box m = Y_m[a,h] * X_m[a,w]  (rank-1 per a-block)
AND the monotone transform x = inter/(areaA+areaB): since iou = x/(1-x),
comparing x across boxes is equivalent to comparing IoU, and the division by
apb folds into host-prescaled X'-tables. The whole 4.7M-pair match collapses
to one block-diagonal PE matmul per box (three bank-aligned chunks), a
Scalar-engine PSUM->SBUF copy, a DVE pack [x-bits | (31-m)<<2], and a DVE
running max. Thresholds: iou>=0.5 <=> x>=1/3 ; iou<0.3 <=> x<3/13.

Payload gather: per box, bf16 eq-compare + two copy_predicated writes of
host-packed u32 payload words ((cx*64)<<16|(cy*64),
(lgw*4096)<<17|(lgh*4096)<<2|label).

Hard-negative mining ranks raw pobj (softplus is monotone); a 12-iteration
binary search (bf16 counts) + second-order tie correction replaces exact
top-k. Partition reductions are ones-column PE matmuls.

Program emits pair(img0), pair(img1), then downstream(img0), downstream(img1)
so the DVE-heavy downstream overlaps the other image's PE/Act pair work.
"""
import numpy as np

import concourse.bass as bass
import concourse.bacc as bacc
import concourse.mybir as mybir
from concourse.tile import TileContext
from concourse.bass_utils import run_bass_kernel_spmd

F32 = mybir.dt.float32
F32R = mybir.dt.float32r
BF16 = mybir.dt.bfloat16
U16 = mybir.dt.uint16
I32 = mybir.dt.int32
U32 = mybir.dt.uint32
OP = mybir.AluOpType
AF = mybir.ActivationFunctionType

B, C, A, H, W, M = 16, 3, 9, 128, 128, 32
K = 5 + C
N_CORES = 8
N_IMG = B // N_CORES
AW = A * W
Hp = H

PK_MASK = 0xFFFFFF80
IDX_MASK = 0x7C
POS_TH = float(np.float32(1.0) / np.float32(3.0))
NEG_TH = float(np.float32(3.0) / np.float32(13.0))
NEG_BIG = 100.0
N_SEARCH = 10
SEARCH_SPAN = 8.0
USE_F32R = True
CH = ((4, 0, 512), (4, 512, 512), (1, 1024, 128))


def build_nc(n_img=N_IMG, debug=False):
    nc = bacc.Bacc("TRN2", target_bir_lowering=False, debug=False)
    d_pred = nc.declare_dram_parameter("pred", [n_img, A * K, H, W], F32, isOutput=False)
    d_planes = nc.declare_dram_parameter("planes", [6, Hp, AW], F32, isOutput=False)
    d_rhs = nc.declare_dram_parameter("rhs", [n_img, 8, 3, 4, 4 * 512], F32, isOutput=False)
    d_yst = nc.declare_dram_parameter("yst", [n_img, 8, 3, 4, 4 * 128], F32, isOutput=False)
    d_pxy = nc.declare_dram_parameter("pxy", [n_img, Hp, M], U32, isOutput=False)
    d_pwh = nc.declare_dram_parameter("pwh", [n_img, Hp, M], U32, isOutput=False)
    d_res = nc.declare_dram_parameter("res", [8, 1], F32, isOutput=True)
    d_dbg = None
    if debug:
        d_dbg = nc.declare_dram_parameter("dbg", [4, Hp, AW], F32, isOutput=True)

    V, G, S, T = nc.vector, nc.gpsimd, nc.scalar, nc.tensor

    with TileContext(nc) as tc:
        with (
            tc.tile_pool(name="persist", bufs=1) as pq,
            tc.tile_pool(name="pred_pool", bufs=1) as pp,
            tc.tile_pool(name="tab_pool", bufs=2) as ptab,
            tc.tile_pool(name="scr_pool", bufs=4) as ps,
            tc.tile_pool(name="img_pool", bufs=1) as pi,
            tc.tile_pool(name="tiny_pool", bufs=2) as pt,
            tc.psum_pool(name="psI", bufs=2) as pmi,
            tc.psum_pool(name="psS", bufs=2) as pms,
        ):
            def scrA(nm="a"):
                return ps.tile([Hp, AW], F32, tag="tmpA", name=nm, bufs=3)

            def scrB(nm="b"):
                return ps.tile([Hp, AW], F32, tag="tmpB", name=nm, bufs=3)

            def scrU(nm="u"):
                return ps.tile([Hp, AW], U32, tag="tmpU", name=nm, bufs=2)

            def junk():
                return ps.tile([Hp, AW], F32, tag="junkN", name="junkN", bufs=1)

            def junkb():
                return ps.tile([Hp, AW], BF16, tag="junkB", name="junkB", bufs=1)

            def smps(nm="sm"):
                return pms.tile([Hp, 1], F32, tag="sm", name=nm)

            def tiny(tag):
                return pt.tile([1, 1], F32, tag=tag, name=tag)

            invaw64 = pq.tile([Hp, AW], F32)
            invah64 = pq.tile([Hp, AW], F32)
            axw = pq.tile([Hp, AW], F32)
            ayh = pq.tile([Hp, AW], F32)
            logaw = pq.tile([Hp, AW], F32)
            logah = pq.tile([Hp, AW], F32)
            for j, t in enumerate((invaw64, invah64, axw, ayh, logaw, logah)):
                nc.sync.dma_start(out=t[:, :], in_=d_planes[j])

            ones_col = pq.tile([Hp, 1], F32)
            V.memset(ones_col[:, :], 1.0)
            ones_row = pq.tile([1, 128], F32)
            V.memset(ones_row[:, :], 1.0)
            res_cols = pq.tile([Hp, 8], F32)
            V.memset(res_cols[:, :], 0.0)

            def preduce(col, nm):
                outp = smps(nm)
                T.matmul(out=outp[0:1, 0:1], lhsT=col[:, 0:1], rhs=ones_col[:, 0:1])
                return outp

            # ================= pair stages (both images) =================
            bmax_tiles = []
            for i in range(n_img):
                bmax = pi.tile([Hp, AW], F32, tag="bmax", name="bmax", bufs=2)
                for g in range(8):
                    rhs_st = ptab.tile([Hp, 4 * 512], F32, tag="rhs_st", name="rhs_st")
                    yst_st = ptab.tile([Hp, 4 * 128], F32, tag="yst_st", name="yst_st")
                    for c in range(3):
                        nc.sync.dma_start(out=rhs_st[32 * c:32 * c + 4, :],
                                          in_=d_rhs[i, g, c])
                        nc.sync.dma_start(out=yst_st[32 * c:32 * c + 4, :],
                                          in_=d_yst[i, g, c])
                    if USE_F32R:
                        rhs_u = ptab.tile([Hp, 4 * 512], F32R, tag="rhs_r",
                                          name="rhs_r", bufs=1)
                        yst_u = ptab.tile([Hp, 4 * 128], F32R, tag="yst_r",
                                          name="yst_r", bufs=1)
                        S.activation(out=rhs_u[0:68, :], in_=rhs_st[0:68, :],
                                     func=AF.Copy)
                        S.activation(out=yst_u[0:68, :], in_=yst_st[0:68, :],
                                     func=AF.Copy)
                    else:
                        rhs_u, yst_u = rhs_st, yst_st
                    for j in range(4):
                        m = 4 * g + j
                        x_ps = pmi.tile([Hp, AW], F32, tag="x_ps", name="x_ps")
                        for c, (k_i, coff, cw) in enumerate(CH):
                            po = 32 * c
                            T.matmul(out=x_ps[:, coff:coff + cw],
                                     lhsT=yst_u[po:po + k_i, 128 * j:128 * j + 128],
                                     rhs=rhs_u[po:po + k_i, 512 * j:512 * j + cw])
                        xs = scrA("xs")
                        S.activation(out=xs[:, :], in_=x_ps[:, :], func=AF.Copy)
                        if m == 0:
                            V.tensor_scalar(out=bmax[:, :].bitcast(U32),
                                            in0=xs[:, :].bitcast(U32),
                                            scalar1=PK_MASK, scalar2=(31 - m) << 2,
                                            op0=OP.bitwise_and, op1=OP.bitwise_or)
                        else:
                            pk = scrU("pk")
                            V.tensor_scalar(out=pk[:, :], in0=xs[:, :].bitcast(U32),
                                            scalar1=PK_MASK, scalar2=(31 - m) << 2,
                                            op0=OP.bitwise_and, op1=OP.bitwise_or)
                            V.tensor_tensor(out=bmax[:, :], in0=bmax[:, :],
                                            in1=pk[:, :].bitcast(F32), op=OP.max)
                bmax_tiles.append(bmax)

            # ================= downstream (both images) =================
            for i in range(n_img):
                bmax = bmax_tiles[i]
                pxyc = pi.tile([Hp, M], U32, tag="pxyc", name="pxyc")
                pwhc = pi.tile([Hp, M], U32, tag="pwhc", name="pwhc")
                nc.sync.dma_start(out=pxyc[:, :], in_=d_pxy[i])
                nc.sync.dma_start(out=pwhc[:, :], in_=d_pwh[i])

                pred_t = pp.tile([Hp, K * A * W], F32, tag="pred", name="pred_t")
                nc.sync.dma_start(out=pred_t[:, :], in_=d_pred[i].transpose([1, 0, 2]))
                pv = pred_t.rearrange("p (a k w) -> p a k w", a=A, k=K)

                # ---------- pred-only prep (overlaps gather on DVE) ----------
                pc = [pv[:, :, 5 + c, :] for c in range(C)]
                pobj = pv[:, :, 4, :]
                ex0 = scrA("ex0")
                S.activation(out=ex0[:, :], in_=pc[0], func=AF.Exp)
                ex1 = scrA("ex1")
                S.activation(out=ex1[:, :], in_=pc[1], func=AF.Exp)
                ex2 = scrA("ex2")
                S.activation(out=ex2[:, :], in_=pc[2], func=AF.Exp)
                es01 = scrB("es01")
                G.tensor_tensor(out=es01[:, :], in0=ex0[:, :], in1=ex1[:, :],
                                op=OP.add)
                es = scrB("es")
                G.tensor_tensor(out=es[:, :], in0=es01[:, :], in1=ex2[:, :],
                                op=OP.add)
                lse = pi.tile([Hp, AW], F32, tag="lse", name="lse")
                S.activation(out=lse[:, :], in_=es[:, :], func=AF.Ln)
                axp = scrB("axp")
                S.activation(out=axp[:, :], in_=pobj, func=AF.Abs)
                exn = scrB("exn")
                S.activation(out=exn[:, :], in_=axp[:, :], func=AF.Exp, scale=-1.0)
                lgp = scrB("lgp")
                S.activation(out=lgp[:, :], in_=exn[:, :], func=AF.Ln,
                             bias=ones_col[:, 0:1])
                rlp = scrB("rlp")
                S.activation(out=rlp[:, :], in_=pobj, func=AF.Relu)
                sp = pi.tile([Hp, AW], F32, tag="sp", name="sp")
                G.tensor_tensor(out=sp[:, :], in0=rlp[:, :], in1=lgp[:, :], op=OP.add)

                # ---------- masks + idx ----------
                bq = scrU("bq")
                V.tensor_scalar(out=bq[:, :], in0=bmax[:, :].bitcast(U32),
                                scalar1=PK_MASK, scalar2=None, op0=OP.bitwise_and)
                posf = pi.tile([Hp, AW], F32, tag="posf", name="posf")
                V.tensor_scalar(out=posf[:, :], in0=bq[:, :].bitcast(F32),
                                scalar1=POS_TH, scalar2=None, op0=OP.is_ge)
                negf = pi.tile([Hp, AW], F32, tag="negf", name="negf")
                V.tensor_scalar(out=negf[:, :], in0=bq[:, :].bitcast(F32),
                                scalar1=NEG_TH, scalar2=None, op0=OP.is_lt)
                idq = scrU("idq")
                V.tensor_scalar(out=idq[:, :], in0=bmax[:, :].bitcast(U32),
                                scalar1=IDX_MASK, scalar2=None, op0=OP.bitwise_and)
                idxb = pi.tile([Hp, AW], BF16, tag="idxb", name="idxb")
                V.tensor_copy(out=idxb[:, :], in_=idq[:, :])
                if debug and i == 0:
                    nc.sync.dma_start(out=d_dbg[0], in_=bmax[:, :])
                    nc.sync.dma_start(out=d_dbg[1], in_=posf[:, :])

                # ---------- payload gather ----------
                accxy = pi.tile([Hp, AW], U32, tag="accxy", name="accxy")
                accwh = pi.tile([Hp, AW], U32, tag="accwh", name="accwh")
                V.memset(accxy[:, :], 0)
                V.memset(accwh[:, :], 0)
                for m in range(M):
                    eqf = ps.tile([Hp, AW], U16, tag="eqf", name="eqf", bufs=2)
                    V.tensor_scalar(out=eqf[:, :], in0=idxb[:, :],
                                    scalar1=float((31 - m) << 2), scalar2=None,
                                    op0=OP.is_equal)
                    V.copy_predicated(accxy[:, :], eqf[:, :],
                                      pxyc[:, m:m + 1].broadcast_to([Hp, AW]))
                    V.copy_predicated(accwh[:, :], eqf[:, :],
                                      pwhc[:, m:m + 1].broadcast_to([Hp, AW]))
                if debug and i == 0:
                    nc.sync.dma_start(out=d_dbg[2], in_=accxy[:, :].bitcast(F32))
                    nc.sync.dma_start(out=d_dbg[3], in_=accwh[:, :].bitcast(F32))
                dbg_hook = debug and i == 0

                # ---------- localization (unpack per dim) ----------
                def unpack(src, sh, mask, nm):
                    u = scrU(f"u_{nm}")
                    if sh > 0:
                        V.tensor_scalar(out=u[:, :], in0=src[:, :], scalar1=sh,
                                        scalar2=None, op0=OP.logical_shift_right)
                        if mask is not None:
                            V.tensor_scalar(out=u[:, :], in0=u[:, :], scalar1=mask,
                                            scalar2=None, op0=OP.bitwise_and)
                    else:
                        V.tensor_scalar(out=u[:, :], in0=src[:, :], scalar1=mask,
                                        scalar2=None, op0=OP.bitwise_and)
                    f = ps.tile([Hp, AW], F32, tag="gq", name=nm, bufs=1)
                    V.tensor_copy(out=f[:, :], in_=u[:, :])
                    return f

                lacc = ps.tile([Hp, AW], F32, tag="lacc", name="lacc", bufs=1)
                for di, (gsrc, invp, offp, dch, sc) in enumerate((
                        ((accxy, 16, None, "gxf"), invaw64, axw, 0, None),
                        ((accxy, 0, 0xFFFF, "gyf"), invah64, ayh, 1, None),
                        ((accwh, 17, None, "qwf"), logaw, None, 2, 1.0 / 4096.0),
                        ((accwh, 2, 0x7FFF, "qhf"), logah, None, 3, 1.0 / 4096.0))):
                    gf = unpack(*gsrc)
                    pch = pv[:, :, dch, :]
                    d = scrB("d")
                    if offp is not None:
                        m1 = scrB("m1")
                        G.tensor_tensor(out=m1[:, :], in0=gf[:, :], in1=invp[:, :],
                                        op=OP.mult)
                        pa = scrB("pa")
                        G.tensor_tensor(out=pa[:, :], in0=pch, in1=offp[:, :],
                                        op=OP.add)
                        G.tensor_tensor(out=d[:, :], in0=pa[:, :], in1=m1[:, :],
                                        op=OP.subtract)
                    else:
                        pl = scrB("pl")
                        G.tensor_tensor(out=pl[:, :], in0=pch, in1=invp[:, :],
                                        op=OP.add)
                        V.scalar_tensor_tensor(out=d[:, :], in0=gf[:, :], scalar=sc,
                                               in1=pl[:, :], op0=OP.mult,
                                               op1=OP.subtract)
                    absd = scrB("absd")
                    S.activation(out=absd[:, :], in_=d[:, :], func=AF.Abs)
                    mm = scrB("mm")
                    V.tensor_scalar_min(out=mm[:, :], in0=absd[:, :], scalar1=1.0)
                    r = scrB("r")
                    V.scalar_tensor_tensor(out=r[:, :], in0=mm[:, :], scalar=-0.5,
                                           in1=absd[:, :], op0=OP.mult, op1=OP.add)
                    cc = ps.tile([Hp, AW], F32, tag="ccT", name="ccT", bufs=2)
                    G.tensor_tensor(out=cc[:, :], in0=mm[:, :], in1=r[:, :],
                                    op=OP.mult)
                    if di == 0:
                        V.tensor_copy(out=lacc[:, :], in_=cc[:, :])
                    else:
                        G.tensor_tensor(out=lacc[:, :], in0=lacc[:, :],
                                        in1=cc[:, :], op=OP.add)
                loc_col = pt.tile([Hp, 1], F32, tag="loc_col", name="loc_col")
                V.scalar_tensor_tensor(out=junk()[:, :], in0=lacc[:, :], scalar=1.0,
                                       in1=posf[:, :], op0=OP.mult, op1=OP.mult,
                                       accum_out=loc_col[:, :])

                # ---------- classification ----------
                tgu = scrU("tgu")
                V.tensor_scalar(out=tgu[:, :], in0=accwh[:, :], scalar1=0x3,
                                scalar2=None, op0=OP.bitwise_and)
                tgtf = scrA("tgtf")
                V.tensor_copy(out=tgtf[:, :], in_=tgu[:, :])
                eq1 = scrA("eq1")
                V.tensor_scalar(out=eq1[:, :], in0=tgtf[:, :], scalar1=1.0,
                                scalar2=None, op0=OP.is_equal)
                eq2 = scrA("eq2")
                V.tensor_scalar(out=eq2[:, :], in0=tgtf[:, :], scalar1=2.0,
                                scalar2=None, op0=OP.is_equal)
                d1 = scrB("d1")
                G.tensor_tensor(out=d1[:, :], in0=pc[1], in1=pc[0], op=OP.subtract)
                d2 = scrB("d2")
                G.tensor_tensor(out=d2[:, :], in0=pc[2], in1=pc[0], op=OP.subtract)
                z1 = scrB("z1")
                V.tensor_tensor(out=z1[:, :], in0=eq1[:, :], in1=d1[:, :], op=OP.mult)
                z2 = scrB("z2")
                G.tensor_tensor(out=z2[:, :], in0=eq2[:, :], in1=d2[:, :], op=OP.mult)
                zz = scrB("zz")
                G.tensor_tensor(out=zz[:, :], in0=z1[:, :], in1=z2[:, :], op=OP.add)
                u1 = scrB("u1")
                G.tensor_tensor(out=u1[:, :], in0=lse[:, :], in1=pc[0],
                                op=OP.subtract)
                clsper = scrB("clsper")
                G.tensor_tensor(out=clsper[:, :], in0=u1[:, :], in1=zz[:, :],
                                op=OP.subtract)
                if dbg_hook:
                    nc.sync.dma_start(out=d_dbg[0], in_=lse[:, :])
                    nc.sync.dma_start(out=d_dbg[1], in_=clsper[:, :])
                cls_col = pt.tile([Hp, 1], F32, tag="cls_col", name="cls_col")
                V.scalar_tensor_tensor(out=junk()[:, :], in0=clsper[:, :], scalar=1.0,
                                       in1=posf[:, :], op0=OP.mult, op1=OP.mult,
                                       accum_out=cls_col[:, :])

                # ---------- objectness ----------
                opos_col = pt.tile([Hp, 1], F32, tag="opos_col", name="opos_col")
                V.scalar_tensor_tensor(out=junk()[:, :], in0=pobj, scalar=1.0,
                                       in1=posf[:, :], op0=OP.mult, op1=OP.mult,
                                       accum_out=opos_col[:, :])
                possp_col = pt.tile([Hp, 1], F32, tag="possp_col", name="possp_col")
                V.scalar_tensor_tensor(out=junk()[:, :], in0=sp[:, :], scalar=1.0,
                                       in1=posf[:, :], op0=OP.mult, op1=OP.mult,
                                       accum_out=possp_col[:, :])
                npos_col = pt.tile([Hp, 1], F32, tag="npos_col", name="npos_col")
                V.tensor_scalar(out=junk()[:, :], in0=posf[:, :], scalar1=1.0,
                                scalar2=None, op0=OP.mult, op1=OP.add,
                                accum_out=npos_col[:, :])
                nneg_col = pt.tile([Hp, 1], F32, tag="nneg_col", name="nneg_col")
                V.tensor_scalar(out=junk()[:, :], in0=negf[:, :], scalar1=1.0,
                                scalar2=None, op0=OP.mult, op1=OP.add,
                                accum_out=nneg_col[:, :])
                s1 = scrB("s1z")
                V.scalar_tensor_tensor(out=s1[:, :], in0=pobj, scalar=NEG_BIG,
                                       in1=negf[:, :], op0=OP.add, op1=OP.mult)
                zt = pi.tile([Hp, AW], F32, tag="zt", name="zt")
                V.tensor_scalar(out=zt[:, :], in0=s1[:, :], scalar1=-NEG_BIG,
                                scalar2=None, op0=OP.add)
                ztb = pi.tile([Hp, AW], BF16, tag="ztb", name="ztb")
                V.tensor_copy(out=ztb[:, :], in_=zt[:, :])

                npos_ps = preduce(npos_col, "npos_ps")
                nneg_ps = preduce(nneg_col, "nneg_ps")
                npos_t = tiny("npos_t")
                V.tensor_copy(out=npos_t[:, :], in_=npos_ps[0:1, 0:1])
                nneg_t = tiny("nneg_t")
                V.tensor_copy(out=nneg_t[:, :], in_=nneg_ps[0:1, 0:1])

                np3 = tiny("np3")
                V.tensor_scalar_mul(out=np3[:, :], in0=npos_t[:, :], scalar1=3.0)
                kmin = tiny("kmin")
                V.tensor_tensor(out=kmin[:, :], in0=np3[:, :], in1=nneg_t[:, :],
                                op=OP.min)
                nn10 = tiny("nn10")
                V.tensor_scalar_mul(out=nn10[:, :], in0=nneg_t[:, :], scalar1=0.1)
                nn10i = pt.tile([1, 1], I32, tag="nn10i", name="nn10i")
                V.tensor_copy(out=nn10i[:, :], in_=nn10[:, :])
                nn10f = tiny("nn10f")
                V.tensor_copy(out=nn10f[:, :], in_=nn10i[:, :])
                k2 = tiny("k2")
                V.tensor_scalar_max(out=k2[:, :], in0=nn10f[:, :], scalar1=1.0)
                znn = tiny("znn")
                V.tensor_scalar(out=znn[:, :], in0=nneg_t[:, :], scalar1=0.0,
                                scalar2=None, op0=OP.is_gt)
                k2z = tiny("k2z")
                V.tensor_tensor(out=k2z[:, :], in0=k2[:, :], in1=znn[:, :],
                                op=OP.mult)
                zf = tiny("zf")
                V.tensor_scalar(out=zf[:, :], in0=npos_t[:, :], scalar1=0.0,
                                scalar2=None, op0=OP.is_equal)
                kd = tiny("kd")
                V.tensor_tensor(out=kd[:, :], in0=k2z[:, :], in1=kmin[:, :],
                                op=OP.subtract)
                kzd = tiny("kzd")
                V.tensor_tensor(out=kzd[:, :], in0=zf[:, :], in1=kd[:, :], op=OP.mult)
                kk = tiny("kk")
                V.tensor_tensor(out=kk[:, :], in0=kmin[:, :], in1=kzd[:, :],
                                op=OP.add)

                # ---------- binary search ----------
                th1 = tiny("th1")
                V.memset(th1[:, :], 0.0)
                for it in range(N_SEARCH):
                    s_i = SEARCH_SPAN * (0.5 ** it)
                    thb_ps = smps("thb_ps")
                    T.matmul(out=thb_ps[:, :], lhsT=ones_row[0:1, :], rhs=th1[:, :])
                    cnt_col = pt.tile([Hp, 1], F32, tag="cnt_col", name="cnt_col")
                    V.tensor_scalar(out=junkb()[:, :], in0=ztb[:, :],
                                    scalar1=thb_ps[:, 0:1], scalar2=None,
                                    op0=OP.is_gt, op1=OP.add,
                                    accum_out=cnt_col[:, :])
                    cnt_ps = preduce(cnt_col, "cnt_ps")
                    ge = tiny("ge")
                    V.tensor_tensor(out=ge[:, :], in0=cnt_ps[0:1, 0:1], in1=kk[:, :],
                                    op=OP.is_ge)
                    V.scalar_tensor_tensor(out=th1[:, :], in0=ge[:, :],
                                           scalar=2.0 * s_i, in1=th1[:, :],
                                           op0=OP.mult, op1=OP.add)
                    V.tensor_scalar_sub(out=th1[:, :], in0=th1[:, :], scalar1=s_i)

                thb_ps = smps("thb_ps")
                T.matmul(out=thb_ps[:, :], lhsT=ones_row[0:1, :], rhs=th1[:, :])
                cntF_col = pt.tile([Hp, 1], F32, tag="cntF_col", name="cntF_col")
                V.tensor_scalar(out=junk()[:, :], in0=zt[:, :],
                                scalar1=thb_ps[:, 0:1], scalar2=None,
                                op0=OP.is_gt, op1=OP.add, accum_out=cntF_col[:, :])
                selsum_col = pt.tile([Hp, 1], F32, tag="selsum_col", name="selsum_col")
                V.scalar_tensor_tensor(out=junk()[:, :], in0=zt[:, :],
                                       scalar=thb_ps[:, 0:1], in1=sp[:, :],
                                       op0=OP.is_gt, op1=OP.mult,
                                       accum_out=selsum_col[:, :])
                cntF_ps = preduce(cntF_col, "cntF_ps")
                cntF_t = tiny("cntF_t")
                V.tensor_copy(out=cntF_t[:, :], in_=cntF_ps[0:1, 0:1])

                tha = tiny("tha")
                S.activation(out=tha[:, :], in_=th1[:, :], func=AF.Abs)
                the = tiny("the")
                S.activation(out=the[:, :], in_=tha[:, :], func=AF.Exp, scale=-1.0)
                thl = tiny("thl")
                S.activation(out=thl[:, :], in_=the[:, :], func=AF.Ln,
                             bias=ones_col[0:1, 0:1])
                thr = tiny("thr")
                S.activation(out=thr[:, :], in_=th1[:, :], func=AF.Relu)
                sth = tiny("sth")
                V.tensor_tensor(out=sth[:, :], in0=thr[:, :], in1=thl[:, :],
                                op=OP.add)
                kc = tiny("kc")
                V.tensor_tensor(out=kc[:, :], in0=kk[:, :], in1=cntF_t[:, :],
                                op=OP.subtract)
                kcs = tiny("kcs")
                V.tensor_tensor(out=kcs[:, :], in0=kc[:, :], in1=sth[:, :],
                                op=OP.mult)

                # ---------- accumulate result columns ----------
                oc = pt.tile([Hp, 1], F32, tag="oc", name="oc")
                V.tensor_tensor(out=oc[:, :], in0=possp_col[:, :],
                                in1=selsum_col[:, :], op=OP.add)
                oc2 = pt.tile([Hp, 1], F32, tag="oc2", name="oc2")
                V.tensor_tensor(out=oc2[:, :], in0=oc[:, :], in1=opos_col[:, :],
                                op=OP.subtract)
                V.tensor_tensor(out=oc2[0:1, :], in0=oc2[0:1, :], in1=kcs[:, :],
                                op=OP.add)
                V.tensor_tensor(out=res_cols[:, 0:1], in0=res_cols[:, 0:1],
                                in1=oc2[:, :], op=OP.add)
                V.tensor_tensor(out=res_cols[:, 1:2], in0=res_cols[:, 1:2],
                                in1=cls_col[:, :], op=OP.add)
                V.tensor_tensor(out=res_cols[:, 2:3], in0=res_cols[:, 2:3],
                                in1=loc_col[:, :], op=OP.add)
                V.tensor_tensor(out=res_cols[:, 3:4], in0=res_cols[:, 3:4],
                                in1=npos_col[:, :], op=OP.add)
                nsc = pt.tile([Hp, 1], F32, tag="nsc", name="nsc")
                V.tensor_copy(out=nsc[:, :], in_=npos_col[:, :])
                V.tensor_tensor(out=nsc[0:1, :], in0=nsc[0:1, :], in1=kk[:, :],
                                op=OP.add)
                V.tensor_tensor(out=res_cols[:, 4:5], in0=res_cols[:, 4:5],
                                in1=nsc[:, :], op=OP.add)

            # ---------- final reduction + output ----------
            res_ps = smps("res_ps")
            T.matmul(out=res_ps[0:8, 0:1], lhsT=res_cols[:, :],
                     rhs=ones_col[:, 0:1])
            res_sb = pt.tile([8, 1], F32, tag="res_sb", name="res_sb")
            V.tensor_copy(out=res_sb[:, :], in_=res_ps[0:8, 0:1])
            nc.sync.dma_start(out=d_res[:, :], in_=res_sb[:, :])

    nc.compile()
    return nc


def prep_inputs(pred, anchors, gt_boxes, gt_labels, n_img=N_IMG):
    """Host-side sharding + separable-table prep (numpy, small)."""
    pred = np.ascontiguousarray(pred, dtype=np.float32)
    anchors = np.asarray(anchors, dtype=np.float32)
    gt_boxes = np.asarray(gt_boxes, dtype=np.float32)
    gt_labels = np.asarray(gt_labels)

    anc = anchors.reshape(H, W, A, 4)
    xt1 = np.ascontiguousarray(anc[0, :, :, 0].T)  # [A, W]
    xt2 = np.ascontiguousarray(anc[0, :, :, 2].T)
    yt1 = np.ascontiguousarray(anc[:, 0, :, 1].T)  # [A, H]
    yt2 = np.ascontiguousarray(anc[:, 0, :, 3].T)

    aw = np.maximum(xt2 - xt1, 1e-6)
    ah = np.maximum(yt2 - yt1, 1e-6)
    invaw = 1.0 / aw
    invah = 1.0 / ah
    acx = (xt1 + xt2) * 0.5
    acy = (yt1 + yt2) * 0.5
    areaA_w = (xt2 - xt1) * (yt2 - yt1)[:, 0:1]    # [A, W] (area const over h)

    def xplane(t):
        return np.broadcast_to(t.reshape(1, A * W), (H, A * W))

    def yplane(t):
        return np.broadcast_to(t.T.reshape(H, A, 1), (H, A, W)).reshape(H, A * W)

    planes = np.ascontiguousarray(np.stack([
        xplane((invaw / 64.0).astype(np.float32)),
        yplane((invah / 64.0).astype(np.float32)),
        xplane((acx * invaw).astype(np.float32)),
        yplane((acy * invah).astype(np.float32)),
        xplane(np.log(aw).astype(np.float32)),
        yplane(np.log(ah).astype(np.float32)),
    ]).astype(np.float32))

    in_maps = []
    n_cores = pred.shape[0] // n_img
    for c in range(n_cores):
        sl = slice(c * n_img, (c + 1) * n_img)
        gtb = gt_boxes[sl]
        gtl = gt_labels[sl]
        rhs = np.zeros((n_img, 8, 3, 4, 4, 512), np.float32)
        yst = np.zeros((n_img, 8, 3, 4, 4, 128), np.float32)
        pxy = np.zeros((n_img, H, M), np.uint32)
        pwh = np.zeros((n_img, H, M), np.uint32)
        for ii in range(n_img):
            bx = gtb[ii]
            gx1, gy1, gx2, gy2 = bx[:, 0], bx[:, 1], bx[:, 2], bx[:, 3]
            areaB = (gx2 - gx1) * (gy2 - gy1)
            Xt = np.maximum(np.minimum(xt2[None], gx2[:, None, None])
                            - np.maximum(xt1[None], gx1[:, None, None]), 0.0
                            ).astype(np.float32)
            Yt = np.maximum(np.minimum(yt2[None], gy2[:, None, None])
                            - np.maximum(yt1[None], gy1[:, None, None]), 0.0
                            ).astype(np.float32)
            for m in range(M):
                g, j = divmod(m, 4)
                # X' = X / (areaA + areaB): folds the union division in
                Xs = (Xt[m] / (areaA_w + areaB[m])).astype(np.float32)  # [A, W]
                for cth in range(3):
                    amax = 4 if cth < 2 else 1
                    for r in range(amax):
                        a = 4 * cth + r
                        rhs[ii, g, cth, r, j, 128 * r:128 * r + 128] = Xs[a]
                        yst[ii, g, cth, r, j, :] = Yt[m, a]
            gcx = (gx1 + gx2) * 0.5
            gcy = (gy1 + gy2) * 0.5
            lgw = np.log(np.maximum(gx2 - gx1, 1e-6))
            lgh = np.log(np.maximum(gy2 - gy1, 1e-6))
            qx = np.round(gcx * 64.0).astype(np.uint32)
            qy = np.round(gcy * 64.0).astype(np.uint32)
            qw = np.round(lgw * 4096.0).astype(np.uint32)
            qh = np.round(lgh * 4096.0).astype(np.uint32)
            lab = np.clip(gtl[ii].astype(np.int64) - 1, 0, C - 1).astype(np.uint32)
            pxy[ii, :, :] = ((qx << np.uint32(16)) | qy)[None, :]
            pwh[ii, :, :] = ((qw << np.uint32(17)) | (qh << np.uint32(2)) | lab)[None, :]
        in_maps.append({
            "pred": np.ascontiguousarray(pred[sl]),
            "planes": planes,
            "rhs": rhs.reshape(n_img, 8, 3, 4, 4 * 512),
            "yst": yst.reshape(n_img, 8, 3, 4, 4 * 128),
            "pxy": pxy,
            "pwh": pwh,
        })
    return in_maps


def finalize(partials):
    tot = np.sum(np.stack([np.asarray(p).reshape(8) for p in partials]),
                 axis=0, dtype=np.float64)
    obj_s, cls_s, loc_s, total_pos, total_sel = tot[:5]
    obj_s, cls_s, loc_s = np.float32(obj_s), np.float32(cls_s), np.float32(loc_s)
    denom_pos = np.float32(max(total_pos, 1.0))
    denom_obj = np.float32(max(total_sel, 1.0))
    loss_loc = np.float32(loc_s / denom_pos)
    loss_cls = np.float32(cls_s / denom_pos)
    loss_obj = np.float32(obj_s / denom_obj)
    loss_total = np.float32(2.0 * loss_loc + 1.0 * loss_cls + 1.0 * loss_obj)
    return np.array([loss_obj, loss_cls, loss_loc, loss_total], dtype=np.float32)


_NC_CACHE = {}


def _get_nc():
    if "nc" not in _NC_CACHE:
        _NC_CACHE["nc"] = build_nc()
    return _NC_CACHE["nc"]


def run_with_results(pred, anchors, gt_boxes, gt_labels, trace=False, **kw):
    nc = _get_nc()
    in_maps = prep_inputs(pred, anchors, gt_boxes, gt_labels)
    res = run_bass_kernel_spmd(nc, in_maps, list(range(N_CORES)), trace=trace, **kw)
    out = finalize([res.results[c]["res"] for c in range(N_CORES)])
    return out, res


def kernel(pred, anchors, gt_boxes, gt_labels):
    return run_with_results(pred, anchors, gt_boxes, gt_labels)[0]
